# revision 2
# baseline (speedup 1.0000x reference)
"""Trainium2 Bass kernel: batched dense attention
   out = softmax((x_q Wq^T + bq)(x_k Wk^T + bk)^T / sqrt(E)) (x_v Wv^T + bv)

Sharding: 8 cores = 4 batches x 2 query-row halves. Each core holds the
full K/V of its batch and 2048 query rows (sequence-parallel on Q).

Device layouts (all matmul operands bf16, accumulation fp32 in PSUM):
  - scores are computed TRANSPOSED (keys on partitions) so the exp'd
    probabilities can feed the P@V matmul as the stationary operand
    without any on-device transposes.
  - softmax is unnormalized: no max subtraction (scores ~ N(0,1) for
    this problem's scale, exp is safe in fp32), Z = sum_k exp(s) is
    accumulated with a ones-vector matmul, and 1/Z is applied to the
    accumulated PV output during the PSUM->SBUF copy.
  - host pre-transposes/pre-casts inputs (x^T, W^T as bf16) so the
    TensorEngine does zero transpose work.
  - bv is added at the very end: softmax rows sum to 1, so
    P @ (V + bv) == (P @ V) + bv.
"""

import numpy as np
import ml_dtypes

import concourse.bacc as bacc
import concourse.mybir as mybir
import concourse.tile as tile
from concourse.bass_utils import run_bass_kernel_spmd

B, S, E = 4, 4096, 1024
N_CORES = 8
HQ = 2              # query halves per batch
SQ = S // HQ        # 2048 query rows per core
P = 128             # partitions
ET = E // P         # 8 embed tiles
SKT = S // P        # 32 key tiles
NQ = 256            # attention query-chunk (2 psum out subtiles)
NQT = SQ // NQ      # 8 chunks per core
CH = 512            # projection column chunk
INV_SCALE = 1.0 / float(E) ** 0.5

BF16 = mybir.dt.bfloat16
F32 = mybir.dt.float32
AF = mybir.ActivationFunctionType

_CACHE = {}


def _emit(nc, tc, dram):
    xqT, xkT, xvT, wqT, wkT, wvT, bqr, bkr, bvb, out = dram

    with (
        tc.tile_pool(name="consts", bufs=1) as cpool,
        tc.tile_pool(name="wq", bufs=1) as wqpool,
        tc.tile_pool(name="kv", bufs=1) as kvpool,
    ):
        ones = cpool.tile([P, 1], BF16)
        nc.gpsimd.memset(ones[:], 1.0)
        bq_sb = cpool.tile([P, ET], F32)
        nc.sync.dma_start(bq_sb[:], bqr[:])
        bk_sb = cpool.tile([P, ET], F32)
        nc.sync.dma_start(bk_sb[:], bkr[:])
        bv_sb = cpool.tile([P, E], F32)
        nc.sync.dma_start(bv_sb[:], bvb[:])

        wq_sb = [wqpool.tile([P, E], BF16, tag=f"wq{dt}", name=f"wq{dt}") for dt in range(ET)]
        for dt in range(ET):
            nc.sync.dma_start(wq_sb[dt][:], wqT[dt * P:(dt + 1) * P, :])

        # resident K^T [E, S] and V [S, E] (bf16, 16 MB)
        kT = [kvpool.tile([P, S], BF16, tag=f"kT{et}", name=f"kT{et}") for et in range(ET)]
        vN = [kvpool.tile([P, E], BF16, tag=f"v{sk}", name=f"v{sk}") for sk in range(SKT)]

        # ---------------- Phase 1: K and V projections ----------------
        with (
            tc.tile_pool(name="wkv", bufs=8) as wpool,
            tc.tile_pool(name="xst", bufs=16) as xpool,
            tc.tile_pool(name="psk", bufs=3, space="PSUM") as psk,
            tc.tile_pool(name="psv", bufs=2, space="PSUM") as psv,
        ):
            wk_sb = [wpool.tile([P, E], BF16, tag="w", name="wk") for _ in range(ET)]
            for dt in range(ET):
                nc.sync.dma_start(wk_sb[dt][:], wkT[dt * P:(dt + 1) * P, :])

            # k^T[e, s] = sum_d WkT[d, e] * xkT[d, s]
            for ch in range(S // CH):
                xs = [xpool.tile([P, CH], BF16, tag="x", name="xs") for _ in range(ET)]
                for dt in range(ET):
                    nc.sync.dma_start(
                        xs[dt][:], xkT[dt * P:(dt + 1) * P, ch * CH:(ch + 1) * CH])
                for et in range(ET):
                    ps = psk.tile([P, CH], F32, tag="pk")
                    for dt in range(ET):
                        nc.tensor.matmul(
                            ps[:], wk_sb[dt][:, et * P:(et + 1) * P], xs[dt][:],
                            start=(dt == 0), stop=(dt == ET - 1))
                    nc.vector.tensor_scalar_add(
                        kT[et][:, ch * CH:(ch + 1) * CH], ps[:], bk_sb[:, et:et + 1])

            wv_sb = [wpool.tile([P, E], BF16, tag="w", name="wv") for _ in range(ET)]
            for dt in range(ET):
                nc.sync.dma_start(wv_sb[dt][:], wvT[dt * P:(dt + 1) * P, :])

            # v[s, e] = sum_d xvT[d, s] * WvT[d, e]   (natural layout, no bias)
            for ch in range(S // CH):
                xs = [xpool.tile([P, CH], BF16, tag="x", name="xs") for _ in range(ET)]
                for dt in range(ET):
                    nc.sync.dma_start(
                        xs[dt][:], xvT[dt * P:(dt + 1) * P, ch * CH:(ch + 1) * CH])
                for si in range(CH // P):
                    sk = ch * (CH // P) + si
                    ps = psv.tile([P, E], F32, tag="pv")
                    for dt in range(ET):
                        for nh in range(2):
                            nc.tensor.matmul(
                                ps[:, nh * 512:(nh + 1) * 512],
                                xs[dt][:, si * P:(si + 1) * P],
                                wv_sb[dt][:, nh * 512:(nh + 1) * 512],
                                start=(dt == 0), stop=(dt == ET - 1))
                    nc.vector.tensor_copy(vN[sk][:], ps[:])

        # ---------------- Phase 2: attention ----------------
        with (
            tc.tile_pool(name="qx", bufs=16) as qxpool,
            tc.tile_pool(name="qt", bufs=16) as qtpool,
            tc.tile_pool(name="ep", bufs=4) as eppool,
            tc.tile_pool(name="fin", bufs=4) as finpool,
            tc.tile_pool(name="pss", bufs=2, space="PSUM") as pss,
            tc.tile_pool(name="pso", bufs=2, space="PSUM") as pso,
            tc.tile_pool(name="psz", bufs=2, space="PSUM") as psz,
        ):
            for qc in range(NQT):
                # q^T[e, chunk] = sum_d WqT[d, e] * xqT[d, chunk]  (+ bq)
                xq = [qxpool.tile([P, NQ], BF16, tag="qx", name="xq") for _ in range(ET)]
                for dt in range(ET):
                    nc.sync.dma_start(
                        xq[dt][:], xqT[dt * P:(dt + 1) * P, qc * NQ:(qc + 1) * NQ])
                qt = [qtpool.tile([P, NQ], BF16, tag="qt", name="qt") for _ in range(ET)]
                for et in range(ET):
                    ps = pss.tile([P, NQ], F32, tag="ps")
                    for dt in range(ET):
                        nc.tensor.matmul(
                            ps[:], wq_sb[dt][:, et * P:(et + 1) * P], xq[dt][:],
                            start=(dt == 0), stop=(dt == ET - 1))
                    nc.vector.tensor_scalar_add(qt[et][:], ps[:], bq_sb[:, et:et + 1])

                po = [pso.tile([P, E], F32, tag="po", name="po") for _ in range(2)]
                pz = [psz.tile([P, 1], F32, tag="pz", name="pz") for _ in range(2)]
                for sk in range(SKT):
                    # scores^T[k_tile, chunk] = sum_e kT[e, k_tile] * qT[e, chunk]
                    ps = pss.tile([P, NQ], F32, tag="ps")
                    for et in range(ET):
                        nc.tensor.matmul(
                            ps[:], kT[et][:, sk * P:(sk + 1) * P], qt[et][:],
                            start=(et == 0), stop=(et == ET - 1))
                    ep = eppool.tile([P, NQ], BF16, tag="ep")
                    nc.scalar.activation(ep[:], ps[:], AF.Exp, scale=INV_SCALE)
                    for j in range(2):
                        lhs = ep[:, j * P:(j + 1) * P]
                        nc.tensor.matmul(
                            pz[j][:], lhs, ones[:],
                            start=(sk == 0), stop=(sk == SKT - 1))
                        for nh in range(2):
                            nc.tensor.matmul(
                                po[j][:, nh * 512:(nh + 1) * 512], lhs,
                                vN[sk][:, nh * 512:(nh + 1) * 512],
                                start=(sk == 0), stop=(sk == SKT - 1))
                for j in range(2):
                    zi = finpool.tile([P, 1], F32, tag="zi")
                    nc.vector.reciprocal(zi[:], pz[j][:])
                    ob = finpool.tile([P, E], F32, tag="ob")
                    nc.scalar.activation(ob[:], po[j][:], AF.Copy, scale=zi[:])
                    ob2 = finpool.tile([P, E], F32, tag="ob2")
                    nc.vector.tensor_add(ob2[:], ob[:], bv_sb[:])
                    r = qc * 2 + j
                    nc.sync.dma_start(out[r * P:(r + 1) * P, :], ob2[:])


def _build():
    if "nc" in _CACHE:
        return _CACHE["nc"]
    nc = bacc.Bacc("TRN2", target_bir_lowering=False, debug=False,
                   num_devices=N_CORES)
    dram = (
        nc.dram_tensor("xqT", [E, SQ], BF16, kind="ExternalInput"),
        nc.dram_tensor("xkT", [E, S], BF16, kind="ExternalInput"),
        nc.dram_tensor("xvT", [E, S], BF16, kind="ExternalInput"),
        nc.dram_tensor("wqT", [E, E], BF16, kind="ExternalInput"),
        nc.dram_tensor("wkT", [E, E], BF16, kind="ExternalInput"),
        nc.dram_tensor("wvT", [E, E], BF16, kind="ExternalInput"),
        nc.dram_tensor("bqr", [P, ET], F32, kind="ExternalInput"),
        nc.dram_tensor("bkr", [P, ET], F32, kind="ExternalInput"),
        nc.dram_tensor("bvb", [P, E], F32, kind="ExternalInput"),
        nc.dram_tensor("out", [SQ, E], F32, kind="ExternalOutput"),
    )
    with tile.TileContext(nc) as tc:
        _emit(nc, tc, dram)
    nc.compile()
    _CACHE["nc"] = nc
    return nc


def _prep_in_maps(query, key, value, Wq, bq, Wk, bk, Wv, bv):
    bf = ml_dtypes.bfloat16
    wqT = np.ascontiguousarray(np.asarray(Wq, np.float32).T.astype(bf))
    wkT = np.ascontiguousarray(np.asarray(Wk, np.float32).T.astype(bf))
    wvT = np.ascontiguousarray(np.asarray(Wv, np.float32).T.astype(bf))
    bqr = np.ascontiguousarray(np.asarray(bq, np.float32).reshape(ET, P).T)
    bkr = np.ascontiguousarray(np.asarray(bk, np.float32).reshape(ET, P).T)
    bvb = np.ascontiguousarray(
        np.broadcast_to(np.asarray(bv, np.float32), (P, E)))
    in_maps = []
    for c in range(N_CORES):
        b, h = divmod(c, HQ)
        xq = np.asarray(query[b, h * SQ:(h + 1) * SQ, :], np.float32)
        in_maps.append({
            "xqT": np.ascontiguousarray(xq.T.astype(bf)),
            "xkT": np.ascontiguousarray(np.asarray(key[b], np.float32).T.astype(bf)),
            "xvT": np.ascontiguousarray(np.asarray(value[b], np.float32).T.astype(bf)),
            "wqT": wqT, "wkT": wkT, "wvT": wvT,
            "bqr": bqr, "bkr": bkr, "bvb": bvb,
        })
    return in_maps


def kernel(query, key, value, Wq, bq, Wk, bk, Wv, bv, _run_kwargs=None):
    nc = _build()
    in_maps = _prep_in_maps(query, key, value, Wq, bq, Wk, bk, Wv, bv)
    res = run_bass_kernel_spmd(nc, in_maps, core_ids=list(range(N_CORES)),
                               **(_run_kwargs or {}))
    out = np.empty((B, S, E), np.float32)
    for c in range(N_CORES):
        b, h = divmod(c, HQ)
        out[b, h * SQ:(h + 1) * SQ, :] = res.results[c]["out"]
    if _run_kwargs:
        _CACHE["last_results"] = res
    return out
